# revision 31
# baseline (speedup 1.0000x reference)
"""Trainium2 Bass kernel for nn_CrossAttnBlock (cross-attention block).

Reference computation (per batch b):
  hn  = channelwise-LN(x)                       # LN over C at each voxel
  sn  = LN(s) over feature dim                  # per style token
  q   = Wq' @ hn ; k = Wk' @ sn^T ; v = Wv' @ sn^T
  attn = softmax(q_h^T k_h / sqrt(dh)) per head
  out = v_h @ attn^T ; y = x + Wo @ out + bo

Sharding: 8 cores = 2 batches x 4 D-slices (data parallel over batch and
spatial positions). k/v (tiny) are computed redundantly per core. No
collectives needed.

Per-core layout strategy:
  x_sb   (C=2x128, N=4096)  channels on partitions
  stats  per-position mean/sumsq via skinny matmuls (ones rhs)
  qT     per n-chunk via lhsT=x-chunk matmuls (positions on partitions)
         -> per-position LN fixup: rank-1 (-mu (x) r) in PSUM + rstd
            tensor_scalar -> PE transpose back to q (C, N)
  scores = k_h^T q_h -> scoresT (SN on partitions, n free) in PSUM
  exp    on ACT (FD=2048), PSUM->SBUF
  AV     lhsT = expT chunks, rhs = [v_h^T | 1] -> outT (pos, heads*33)
         col 33h+32 = softmax denominator (per-partition!) -> normalize
         with tensor_scalar, PE-transpose back to (C, N)
  o-proj + residual + DMA out.
"""

import os
import numpy as np
from contextlib import ExitStack

import concourse.bass as bass
import concourse.tile as tile
from concourse import bacc, mybir, bass_utils
from concourse.bass import ts, ds
from concourse.masks import make_identity

# ---- problem constants (hardcoded per the harness contract) ----
B, C, D, W, H = 2, 256, 16, 32, 32
CS, SN, HEADS = 256, 256, 8
DH = C // HEADS          # 32
EPS = 1e-6
NCORES = 8
DSL = D // 4             # D-slice per core = 4
N = DSL * W * H          # positions per core = 4096
NB = 256                 # positions per softmax block
NBLK = N // NB           # 16
SCALE = float(DH) ** -0.5

F32 = mybir.dt.float32
F32R = mybir.dt.float32r
BF16 = mybir.dt.bfloat16

# compute dtype config: "f32", "f32r", "bf16"
COMPUTE = os.environ.get("XATTN_COMPUTE", "bf16")


def _dt_pair():
    """(x-side matmul cast dtype or None, engine-written tensor dtype)."""
    if COMPUTE == "f32":
        return None, F32
    if COMPUTE == "f32r":
        return F32R, F32
    if COMPUTE == "bf16r":
        return F32R, BF16
    return None, BF16


def build_kernel(use_qb: bool, use_bo: bool):
    """Build the SPMD single-core Bass program (same program on all cores)."""
    DTX, DT = _dt_pair()
    nc = bacc.Bacc("TRN2", target_bir_lowering=False, debug=False,
                   num_devices=NCORES)

    XDT = F32 if DTX is None else DTX
    xs = nc.dram_tensor("xs", (C, N), XDT, kind="ExternalInput").ap()
    sb = nc.dram_tensor("sb", (SN, CS), F32, kind="ExternalInput").ap()
    aq = nc.dram_tensor("aq", (C, C), XDT, kind="ExternalInput").ap()
    ak = nc.dram_tensor("ak", (CS, C), DT, kind="ExternalInput").ap()
    av = nc.dram_tensor("av", (CS, C), DT, kind="ExternalInput").ap()
    ao = nc.dram_tensor("ao", (C, C), DT, kind="ExternalInput").ap()
    rrow = nc.dram_tensor("rrow", (1, C), F32, kind="ExternalInput").ap()
    qbr = nc.dram_tensor("qbr", (1, C), DT, kind="ExternalInput").ap()
    bor = nc.dram_tensor("bor", (1, C), F32, kind="ExternalInput").ap()
    y = nc.dram_tensor("y", (C, N), F32, kind="ExternalOutput").ap()

    with tile.TileContext(nc) as tc, ExitStack() as ctx:
        P = 128
        # view an XDT (possibly f32r) AP as plain f32 for non-matmul use
        xf32 = (lambda ap: ap) if DTX is None else (lambda ap: ap.bitcast(F32))
        singles = ctx.enter_context(tc.tile_pool(name="singles", bufs=1))
        sm = ctx.enter_context(tc.tile_pool(name="sm", bufs=4))
        qtp = ctx.enter_context(tc.tile_pool(name="qtp", bufs=3))
        epool = ctx.enter_context(tc.tile_pool(name="epool", bufs=4))
        onp = ctx.enter_context(tc.tile_pool(name="onp", bufs=3))
        yp = ctx.enter_context(tc.tile_pool(name="yp", bufs=3))
        # PSUM pools: total must fit 8 banks.
        scps = ctx.enter_context(tc.tile_pool(name="scps", bufs=1, space="PSUM"))   # 4 banks
        psa = ctx.enter_context(tc.tile_pool(name="psa", bufs=2, space="PSUM"))     # 2 banks
        psb = ctx.enter_context(tc.tile_pool(name="psb", bufs=2, space="PSUM"))     # 2 banks

        # ---- load constants / weights ----
        ident = singles.tile([P, P], DT)
        make_identity(nc, ident[:])
        ident32 = singles.tile([P, P], F32)
        make_identity(nc, ident32[:])
        ones2 = singles.tile([P, 2], F32)
        nc.vector.memset(ones2[:], 1.0)
        eps_t = singles.tile([P, 1], F32)
        nc.vector.memset(eps_t[:], EPS)
        ones_row = singles.tile([1, 512], F32)
        nc.vector.memset(ones_row[:], 1.0)

        aq_sb = singles.tile([P, 2, C], XDT)
        ak_sb = singles.tile([P, 2, C], DT)
        av_sb = singles.tile([P, 2, C], DT)
        ao_sb = singles.tile([P, 2, C], DT)
        for g in range(2):
            nc.sync.dma_start(aq_sb[:, g, :], aq[ts(g, P), :])
            nc.sync.dma_start(ak_sb[:, g, :], ak[ts(g, P), :])
            nc.sync.dma_start(av_sb[:, g, :], av[ts(g, P), :])
            nc.sync.dma_start(ao_sb[:, g, :], ao[ts(g, P), :])
        rrow_sb = singles.tile([1, C], F32)
        nc.sync.dma_start(rrow_sb[:], rrow[:])
        qb_sb = singles.tile([1, C], DT)
        nc.sync.dma_start(qb_sb[:], qbr[:])
        bo_sb = singles.tile([1, C], F32)
        nc.sync.dma_start(bo_sb[:], bor[:])

        x_sb = singles.tile([P, 2, N], XDT)
        for g in range(2):
            nc.sync.dma_start(x_sb[:, g, :], xs[ts(g, P), :])

        # ---- style-token LN -> sn -> snT -> k, v, vaug ----
        s_sb = singles.tile([P, 2, CS], F32)
        for j in range(2):
            nc.sync.dma_start(s_sb[:, j, :], sb[ts(j, P), :])
        sn_sb = singles.tile([P, 2, CS], DT)
        for j in range(2):
            st6 = sm.tile([P, 6], F32, tag="st6")
            nc.vector.bn_stats(st6[:], s_sb[:, j, :])
            mv = sm.tile([P, 2], F32, tag="mv")
            nc.vector.bn_aggr(mv[:], st6[:])
            sd = sm.tile([P, 1], F32, tag="sd")
            nc.scalar.activation(sd[:], mv[:, 1:2],
                                 mybir.ActivationFunctionType.Sqrt,
                                 bias=eps_t[:], scale=1.0)
            rstd = sm.tile([P, 1], F32, tag="rstd")
            nc.vector.reciprocal(rstd[:], sd[:])
            nc.vector.tensor_scalar(
                out=sn_sb[:, j, :], in0=s_sb[:, j, :],
                scalar1=mv[:, 0:1], scalar2=rstd[:],
                op0=mybir.AluOpType.subtract, op1=mybir.AluOpType.mult)
        snt_sb = singles.tile([P, 2, SN], DT)
        for j in range(2):
            for kk in range(2):
                tp = psb.tile([P, P], DT, tag="tr")
                nc.tensor.transpose(tp[:], sn_sb[:, j, ts(kk, P)], ident[:])
                nc.vector.tensor_copy(snt_sb[:, kk, ts(j, P)], tp[:])

        # k_z: zero-padded per-head keys so each head's scores matmul can
        # contract over the full K=128 partitions (base-partition rule).
        k_z = singles.tile([P, HEADS, SN], DT)
        nc.vector.memset(k_z[:], 0.0)
        v_sb = singles.tile([P, 2, SN], DT)
        for m in range(2):
            kp = psa.tile([P, SN], F32, tag="t")
            for g in range(2):
                nc.tensor.matmul(kp[:], ak_sb[:, g, ts(m, P)], snt_sb[:, g, :],
                                 start=(g == 0), stop=(g == 1))
            for hh in range(4):
                nc.vector.tensor_copy(
                    k_z[ds(DH * hh, DH), 4 * m + hh, :], kp[ds(DH * hh, DH), :])
            vp = psa.tile([P, SN], F32, tag="t")
            for g in range(2):
                nc.tensor.matmul(vp[:], av_sb[:, g, ts(m, P)], snt_sb[:, g, :],
                                 start=(g == 0), stop=(g == 1))
            nc.vector.tensor_copy(v_sb[:, m, :], vp[:])
        # vaug[tt][p, h, 0:32] = v^T slice, [:, h, 32] = 1.0
        vaug = singles.tile([P, 2, HEADS, DH + 1], DT)
        nc.vector.memset(vaug[:, :, :, DH:DH + 1], 1.0)
        for m in range(2):
            for tt in range(2):
                tp = psb.tile([P, P], DT, tag="tr")
                nc.tensor.transpose(tp[:], v_sb[:, m, ts(tt, P)], ident[:])
                nc.vector.tensor_copy(
                    vaug[:, tt, 4 * m:4 * m + 4, 0:DH],
                    tp[:].rearrange("p (h d) -> p h d", h=4))

        # optional: qlogit[t] = sum_c qb[c] k[c, t]  (rank-1 score shift from ln_b)
        if use_qb:
            # qbz: (128, 128) with col h = qb restricted to head h's rows.
            qbz = singles.tile([P, P], DT)
            nc.vector.memset(qbz[:], 0.0)
            for g in range(2):
                tp = psb.tile([P, P], DT, tag="tr")
                nc.tensor.transpose(tp[:, 0:1], qb_sb[:, ts(g, P)], ident[:])
                for hh in range(4):
                    nc.vector.tensor_copy(
                        qbz[ds(DH * hh, DH), 4 * g + hh:4 * g + hh + 1],
                        tp[ds(DH * hh, DH), 0:1])
            # one matmul per head against its zero-padded key tile; row h of
            # the result is this head's logit shift -> flatten via DMA.
            kqb_sb8 = singles.tile([P, HEADS, SN], F32)
            for h in range(HEADS):
                kq1 = psb.tile([P, SN], F32, tag="tr")
                nc.tensor.matmul(kq1[:], qbz[:], k_z[:, h, :],
                                 start=True, stop=True)
                nc.vector.tensor_copy(kqb_sb8[:, h, :], kq1[:])
            kqb_sb = singles.tile([1, HEADS, SN], F32)
            for h in range(HEADS):
                nc.sync.dma_start(kqb_sb[0:1, h, :], kqb_sb8[h:h + 1, h, :])

        # ---- x stats: per-position mean & sumsq via skinny matmuls ----
        # (fp32 matmul moving operand must have even innermost count, so the
        # ones rhs is (128, 2) and each sum lands duplicated in 2 columns.)
        x2_sb = singles.tile([P, 2, N], F32)
        for g in range(2):
            nc.gpsimd.tensor_mul(x2_sb[:, g, :], xf32(x_sb[:, g, :]),
                                 xf32(x_sb[:, g, :]))
        stats_sb = singles.tile([P, 32, 2], F32)
        for j in range(32):
            stp = psb.tile([P, 4], F32, tag="tr")
            for g in range(2):
                nc.tensor.matmul(stp[:, 0:2], xf32(x_sb[:, g, ts(j, P)]),
                                 ones2[:], start=(g == 0), stop=(g == 1))
            for g in range(2):
                nc.tensor.matmul(stp[:, 2:4], x2_sb[:, g, ts(j, P)], ones2[:],
                                 start=(g == 0), stop=(g == 1))
            nc.vector.tensor_copy(stats_sb[:, j, :], stp[:, 0::2])
        mu_all = singles.tile([P, 32], F32)
        nc.vector.tensor_scalar_mul(mu_all[:], stats_sb[:, :, 0], 1.0 / C)
        m2_all = sm.tile([P, 32], F32, tag="m2")
        nc.vector.tensor_scalar_mul(m2_all[:], stats_sb[:, :, 1], 1.0 / C)
        musq = sm.tile([P, 32], F32, tag="musq")
        nc.vector.tensor_mul(musq[:], mu_all[:], mu_all[:])
        var_all = sm.tile([P, 32], F32, tag="var")
        nc.vector.tensor_sub(var_all[:], m2_all[:], musq[:])
        sd_all = sm.tile([P, 32], F32, tag="sdall")
        nc.scalar.activation(sd_all[:], var_all[:],
                             mybir.ActivationFunctionType.Sqrt,
                             bias=eps_t[:], scale=1.0)
        rstd_all = singles.tile([P, 32], F32)
        nc.vector.reciprocal(rstd_all[:], sd_all[:])
        negmu = sm.tile([P, 32], F32, tag="negmu")
        nc.vector.tensor_scalar_mul(negmu[:], mu_all[:], -1.0)
        nmt = psb.tile([32, P], F32, tag="tr")
        nc.tensor.transpose(nmt[:], negmu[:], ident32[:])
        negmu_r32 = singles.tile([32, P], F32)
        nc.vector.tensor_copy(negmu_r32[:], nmt[:])
        # flatten (32, 128) rows into a single-partition (1, 4096) row so the
        # rank-1 lhsT slices sit at base partition 0
        negmu_row = singles.tile([1, N], F32)
        for j in range(32):
            nc.sync.dma_start(negmu_row[0:1, ds(j * P, P)],
                              negmu_r32[j:j + 1, :])

        # ---- qT per n-chunk -> normalize -> transpose to q (C, N) ----
        q_sb = singles.tile([P, 2, N], DT)
        for j in range(32):
            qt = psa.tile([P, C], F32, tag="t")
            for g in range(2):
                nc.tensor.matmul(qt[:], x_sb[:, g, ts(j, P)], aq_sb[:, g, :],
                                 start=(g == 0), stop=False)
            nc.tensor.matmul(qt[:], negmu_row[0:1, ds(j * P, P)], rrow_sb[:],
                             start=False, stop=True)
            qtn = qtp.tile([P, C], DT, tag="qtn")
            nc.vector.tensor_scalar_mul(qtn[:], qt[:], rstd_all[:, j:j + 1])
            for kk in range(2):
                qp = psb.tile([P, P], DT, tag="tr")
                nc.tensor.transpose(qp[:], qtn[:, ts(kk, P)], ident[:])
                nc.vector.tensor_copy(q_sb[:, kk, ts(j, P)], qp[:])

        # ---- attention blocks ----
        out_sb = singles.tile([P, 2, N], DT)
        for nb in range(NBLK):
            e_t = []
            for mt in range(2):
                sc = scps.tile([P, HEADS, NB], F32, tag="sc")
                for h in range(HEADS):
                    nc.tensor.matmul(
                        sc[:, h, :],
                        k_z[:, h, ts(mt, P)],
                        q_sb[:, h // 4, ds(nb * NB, NB)],
                        start=True, stop=not use_qb)
                    if use_qb:
                        nc.tensor.matmul(
                            sc[:, h, :], kqb_sb[0:1, h, ds(mt * P, P)],
                            ones_row[:, 0:NB], start=False, stop=True)
                e = epool.tile([P, HEADS, NB], DT, tag="e")
                nc.scalar.activation(e[:], sc[:],
                                     mybir.ActivationFunctionType.Exp,
                                     scale=SCALE)
                e_t.append(e)
            for u in range(NB // P):
                op = psa.tile([P, HEADS, DH + 1], F32, tag="t")
                for h in range(HEADS):
                    for mt in range(2):
                        nc.tensor.matmul(
                            op[:, h, :],
                            e_t[mt][:, h, ts(u, P)],
                            vaug[:, mt, h, :],
                            start=(mt == 0), stop=(mt == 1))
                rz = sm.tile([P, HEADS], F32, tag="rz")
                nc.vector.reciprocal(rz[:], op[:, :, DH])
                onrm = onp.tile([P, C], DT, tag="onrm")
                nc.vector.tensor_mul(
                    onrm[:].rearrange("p (h d) -> p h d", h=HEADS),
                    op[:, :, 0:DH],
                    rz[:].broadcast_to((P, HEADS, DH)))
                for kk in range(2):
                    otp = psb.tile([P, P], DT, tag="tr")
                    nc.tensor.transpose(otp[:], onrm[:, ts(kk, P)], ident[:])
                    nc.vector.tensor_copy(
                        out_sb[:, kk, ds(nb * NB + u * P, P)], otp[:])

        # ---- output projection + bias + residual ----
        for c8 in range(N // 512):
            for m in range(2):
                fin = psa.tile([P, 512], F32, tag="t")
                for g in range(2):
                    nc.tensor.matmul(fin[:], ao_sb[:, g, ts(m, P)],
                                     out_sb[:, g, ds(c8 * 512, 512)],
                                     start=(g == 0),
                                     stop=(g == 1 and not use_bo))
                if use_bo:
                    nc.tensor.matmul(
                        fin[:], bo_sb[:, ts(m, P)], ones_row[:],
                        start=False, stop=True)
                yt = yp.tile([P, 512], F32, tag="yt")
                nc.vector.tensor_add(yt[:], fin[:],
                                     xf32(x_sb[:, m, ds(c8 * 512, 512)]))
                nc.sync.dma_start(y[ts(m, P), ds(c8 * 512, 512)], yt[:])

    nc.compile()
    return nc


_CACHE = {}


def _get_nc(use_qb, use_bo):
    key = (COMPUTE, use_qb, use_bo)
    if key not in _CACHE:
        _CACHE[key] = build_kernel(use_qb, use_bo)
    return _CACHE[key]


def _prep(inputs):
    f32 = np.float32
    x = np.ascontiguousarray(inputs["x"], dtype=f32)
    s = np.ascontiguousarray(inputs["s"], dtype=f32)
    ln_w, ln_b = inputs["ln_w"].astype(f32), inputs["ln_b"].astype(f32)
    lns_w, lns_b = inputs["lns_w"].astype(f32), inputs["lns_b"].astype(f32)
    Wq, Wk, Wv, Wo = (inputs[k].astype(f32) for k in ("Wq", "Wk", "Wv", "Wo"))
    bo = inputs["bo"].astype(f32)

    A_q = np.ascontiguousarray((Wq * ln_w[None, :]).T)       # (ci, co)
    A_k = np.ascontiguousarray((Wk * lns_w[None, :]).T)      # (ci_s, co)
    A_v = np.ascontiguousarray((Wv * lns_w[None, :]).T)
    A_o = np.ascontiguousarray(Wo.T)
    r = A_q.sum(axis=0, dtype=np.float64).astype(f32)        # rowsums of Wq'
    qb = Wq @ ln_b                                           # q shift from ln_b
    # lns_b: k-shift cancels in softmax; v-shift folds into bo
    bo_eff = bo + Wo @ (Wv @ lns_b)
    use_qb = bool(np.abs(qb).max() > 0)
    use_bo = bool(np.abs(bo_eff).max() > 0)
    return x, s, A_q, A_k, A_v, A_o, r, qb, bo_eff, use_qb, use_bo


def kernel(**inputs):
    x, s, A_q, A_k, A_v, A_o, r, qb, bo_eff, use_qb, use_bo = _prep(inputs)
    nc = _get_nc(use_qb, use_bo)
    DTX, DT = _dt_pair()
    npdt = np.float32 if DT != BF16 else mybir.dt.np(BF16)

    wmap = {
        "aq": A_q.astype(np.float32),
        "ak": A_k.astype(npdt), "av": A_v.astype(npdt), "ao": A_o.astype(npdt),
        "rrow": r.reshape(1, C),
        "qbr": qb.reshape(1, C).astype(npdt),
        "bor": bo_eff.reshape(1, C),
    }
    in_maps = []
    for core in range(NCORES):
        b, dsl = core // 4, core % 4
        xs = np.ascontiguousarray(
            x[b, :, dsl * DSL:(dsl + 1) * DSL].reshape(C, N))
        m = dict(wmap)
        m["xs"] = xs
        m["sb"] = np.ascontiguousarray(s[b])
        in_maps.append(m)

    res = bass_utils.run_bass_kernel_spmd(
        nc, in_maps, core_ids=list(range(NCORES)))
    y = np.empty((B, C, D, W, H), dtype=np.float32)
    for core in range(NCORES):
        b, dsl = core // 4, core % 4
        y[b, :, dsl * DSL:(dsl + 1) * DSL] = \
            res.results[core]["y"].reshape(C, DSL, W, H)
    return y
